# revision 2
# baseline (speedup 1.0000x reference)
"""ArcFace (AngularPenaltySMLoss) on 8 TRN2 NeuronCores.

Sharding (model-parallel softmax): 32768 classes / 8 cores = 4096 per
core (2 PSUM groups x 2048). Host prep is layout/dtype only: fT
[512,2048] fp8, wT [512,4096] bf16 per shard, fnat/wtgt [2048,512] bf16.

Per-core structure (v9; empirical costs: ACT 0.83ns/el+350ns, DVE
bf16-in muls ~0.6ns/el, DVE reduces ~1.1ns/el, broadcast quant muls
~1.1ns/el no-2x, GpSimd TT ~3.4ns/el AND it contends with DVE for the
shared second SBUF port, DR-MM ~216ns per 512-wide at full clock):
  - ACT chain is the bottleneck: Ln/Exp column norms (both groups,
    pre-sweep), rs batches, then 32 sweep exps [128,2048] with
    accum_out row sums (+281ns ACTIVATION_READ_ACCUMULATOR each; DVE
    reduces of the bf16 exp output measure 2.5us/tile - worse).
  - DMA: 1KB rows only (2KB+ rows win HBM bursts unfairly and create
    20-40us cross-core skew that the tail AllGather eats); priority
    wT-g0, wT-g1, fnat01, fT, fnat, wtgt on sync+gpsimd hwdge queues.
    Head DMA is chip-HBM-bound (~34us for 9.5MB x 8 cores).
  - DVE: weight squares + whats fp8 quantize (per-512-chunk APs) +
    ssf/rawdot/wn2 products and reduces. GpSimd does only DMA issue
    and collective triggers (its TT ops would slow DVE ~1.6x).
  - Collectives: 2 AllGathers (sumsA after sweep A, hidden; sumsB at
    the end, ~6-9us flight + laggard wait). Combine math runs under
    the flights, gated behind the last sweep exp via add_dep_helper.
  - All ACT functions in the single natural_log_exp table set (sqrt
    via exp(-0.5*ln)).
"""
import math

import numpy as np
import ml_dtypes

import concourse.bass as bass
import concourse.tile as tile
from concourse import bacc, mybir
from concourse.bass_utils import run_bass_kernel_spmd
from concourse.tile import add_dep_helper

B = 2048
D = 512
C = 32768
NCORES = 8
CS = C // NCORES          # 4096
S = 64.0
MARGIN = 0.5
EPS = 1e-7
COSM = math.cos(MARGIN)
SINM = math.sin(MARGIN)

NB = B // 128             # 16 batch tiles
NK = D // 128             # 4 k-chunks
NG = 2                    # class groups per core
CW = CS // NG             # 2048 classes per group
NH = NB // 2

F32 = mybir.dt.float32
BF16 = mybir.dt.bfloat16
FP8 = mybir.dt.float8e4
FP8NP = ml_dtypes.float8_e4m3fn
BF16NP = ml_dtypes.bfloat16
AF = mybir.ActivationFunctionType
ALU = mybir.AluOpType
DR = mybir.MatmulPerfMode.DoubleRow
AXX = mybir.AxisListType.X

_CACHE = {}

_ONE_SET = "natural_log_exp_and_others"


def _patch_act_tables():
    from concourse import hw_specs, bacc as bacc_mod
    if getattr(bacc_mod, "_act_tables_patched", False):
        return
    orig = hw_specs.get_activation_tables

    def patched(arch):
        t = orig(arch)
        return {name: (funcs if name == _ONE_SET else set())
                for name, funcs in t.items()}

    bacc_mod.get_activation_tables = patched
    bacc_mod._act_tables_patched = True


def _build():
    _patch_act_tables()
    nc = bacc.Bacc(None, target_bir_lowering=False, debug=False)

    fT_ext = nc.declare_dram_parameter("fT", [D, B], FP8, isOutput=False)
    wT_ext = nc.declare_dram_parameter("wT", [D, CS], BF16, isOutput=False)
    fnat_ext = nc.declare_dram_parameter("fnat", [B, D], BF16, isOutput=False)
    wtgt_ext = nc.declare_dram_parameter("wtgt", [B, D], BF16, isOutput=False)
    out_ext = nc.declare_dram_parameter("out", [1, 1], F32, isOutput=True)

    ccA_in = nc.dram_tensor("ccA_in", [128, NB], F32)
    ccA_out = nc.dram_tensor("ccA_out", [128 * NCORES, NB], F32,
                             addr_space="Shared")
    ccB_in = nc.dram_tensor("ccB_in", [128, NB], F32)
    ccB_out = nc.dram_tensor("ccB_out", [128 * NCORES, NB], F32,
                             addr_space="Shared")

    with tile.TileContext(nc) as tc:
        with (
            tc.tile_pool(name="persist", bufs=1) as pp,
            tc.tile_pool(name="prod", bufs=4) as sp,
            tc.tile_pool(name="ejunk", bufs=3) as ep,
        ):
            # ---- persistent SBUF ----
            ft3 = pp.tile([128, NK, B], FP8)
            wt3 = pp.tile([128, NK, CS], BF16)
            whats = [pp.tile([128, NK, 512], FP8, tag=f"what{i}",
                             name=f"what{i}") for i in range(8)]
            fnat3 = pp.tile([128, NB, D], BF16)
            wtgt3 = pp.tile([128, NB, D], BF16)
            ones128 = pp.tile([128, 128], BF16)
            ones_f32 = pp.tile([128, 1], F32)
            sqw = pp.tile([128, NK, CS], BF16)      # squares of wt3
            lnr = pp.tile([128, NG, CW], F32)       # Ln scratch per group
            rnr = pp.tile([128, 8, 512], BF16)      # 1/||w|| per class
            sumsA = pp.tile([128, NB], F32)
            sumsB = pp.tile([128, NB], F32)
            ssf = pp.tile([128, NB], F32)
            rawdot = pp.tile([128, NB], F32)
            wn2 = pp.tile([128, NB], F32)
            rs_pt = pp.tile([128, NB], F32)
            lssf = pp.tile([128, NB], F32)

            # ---- DMA: priority order, two hwdge queues ----
            wTr = wT_ext[:].rearrange("(k p) c -> p k c", p=128)
            fTr = fT_ext[:].rearrange("(k p) b -> p k b", p=128)
            fnr = fnat_ext[:].rearrange("(t p) d -> p t d", p=128)
            wgr = wtgt_ext[:].rearrange("(t p) d -> p t d", p=128)
            qi = 0

            def dma(dst, src):
                nonlocal qi
                eng = nc.sync if qi % 2 == 0 else nc.gpsimd
                qi += 1
                eng.dma_start(dst, src)

            # wT first (heads the whats chain; both groups pre-chain).
            # 1KB rows: 2KB+ rows win bursts unfairly and create 20-40us
            # of cross-core skew that the tail AllGather then eats (the
            # laggard core dominates); 1KB keeps HBM arbitration fair.
            for g in range(NG):
                for q in range(4):
                    cs = slice(g * CW + q * 512, g * CW + q * 512 + 512)
                    dma(wt3[:, :, cs], wTr[:, :, cs])
            dma(fnat3[:, 0:2, :], fnr[:, 0:2, :])
            # fT: [128, 2, 1024] fp8 (1KB rows)
            for kp in range(2):
                for bh in range(2):
                    bs = slice(bh * 1024, bh * 1024 + 1024)
                    dma(ft3[:, 2 * kp:2 * kp + 2, bs],
                        fTr[:, 2 * kp:2 * kp + 2, bs])
            # fnat rest
            for t0 in range(2, NB, 2):
                dma(fnat3[:, t0:t0 + 2, :], fnr[:, t0:t0 + 2, :])
            # wtgt last (needed only from ~35us for rawdot/wn2)
            for t0 in range(0, NB, 2):
                dma(wtgt3[:, t0:t0 + 2, :], wgr[:, t0:t0 + 2, :])

            nc.vector.memset(ones128[:], 1.0)
            nc.vector.memset(ones_f32[:], 1.0)

            pmain_cm = tc.tile_pool(name="pmain", bufs=2, space="PSUM")
            pmain = pmain_cm.__enter__()

            # ---- DVE head: squares (bf16 2x) ----
            def sqw_grp(g):
                for q in range(4):
                    cs = slice(g * CW + q * 512, g * CW + q * 512 + 512)
                    nc.vector.tensor_mul(sqw[:, :, cs], wt3[:, :, cs],
                                         wt3[:, :, cs])

            sqw_grp(0)
            sqw_grp(1)

            # PE: warm-up fillers then norm MMs, both groups pre-chain
            for _ in range(16):
                nc.tensor.ldweights(ones128[:])

            def norm_mms(g, z):
                for q in range(4):
                    cs = slice(g * CW + q * 512, g * CW + q * 512 + 512)
                    for k in range(NK):
                        nc.tensor.matmul(z[:, bass.ts(q, 512)],
                                         ones128[:], sqw[:, k, cs],
                                         start=(k == 0), stop=(k == NK - 1))

            zs0 = pmain.tile([128, CW], F32, tag="z", name="zs0")
            norm_mms(0, zs0)
            zs1 = pmain.tile([128, CW], F32, tag="z", name="zs1")
            norm_mms(1, zs1)

            # ACT: rnr = exp(-0.5 ln(nrm2)), rnr8 layout [128, 8, 512]
            def rnr_acts(g, z):
                nc.scalar.activation(lnr[:, g, :], z[:], AF.Ln)
                nc.scalar.activation(
                    rnr[:, 4 * g:4 * g + 4, :]
                    .rearrange("p a b -> p (a b)"),
                    lnr[:, g, :], AF.Exp, scale=-0.5)

            rnr_acts(0, zs0)
            rnr_acts(1, zs1)

            # DVE: quantize whats (per 512-chunk, baseline-proven 2x AP)
            def quant_grp(g):
                for c4 in range(4):
                    cc = 4 * g + c4
                    nc.vector.tensor_mul(
                        whats[cc][:],
                        wt3[:, :, bass.ts(cc, 512)],
                        rnr[:, cc:cc + 1, :].broadcast_to([128, NK, 512]))

            # DVE: ssf -> rs batches
            def ssf_grp(t0, t1):
                n = t1 - t0
                prod = sp.tile([128, n, D], BF16, tag="prod", name="sprod")
                nc.vector.tensor_mul(prod[:], fnat3[:, t0:t1, :],
                                     fnat3[:, t0:t1, :])
                nc.vector.tensor_reduce(ssf[:, t0:t1], prod[:],
                                        axis=AXX, op=ALU.add)

            def rs_batch(h0, h1):
                nc.scalar.activation(lssf[:, h0:h1], ssf[:, h0:h1], AF.Ln,
                                     scale=1.0 / 4096.0)
                nc.scalar.activation(rs_pt[:, h0:h1], lssf[:, h0:h1], AF.Exp,
                                     scale=-0.5)

            ssf_grp(0, 2)
            rs_batch(0, 2)
            quant_grp(0)

            # ---- sweep helper ----
            def sweep_tile(g, b, sums):
                zp = pmain.tile([128, CW], F32, tag="z", name="zp")
                for j in range(2):
                    for c4 in range(4):
                        cc = 4 * g + c4
                        nc.tensor.matmul(
                            zp[:, bass.ts(c4, 512)],
                            ft3[:, 2 * j:2 * j + 2, bass.ts(b, 128)],
                            whats[cc][:, 2 * j:2 * j + 2, :],
                            start=(j == 0), stop=(j == 1),
                            perf_mode=DR)
                ej = ep.tile([128, CW], BF16, tag="e", name="ej")
                return nc.scalar.activation(ej[:], zp[:], AF.Exp,
                                            scale=rs_pt[:, b:b + 1],
                                            accum_out=sums[:, b:b + 1])

            # sweep A; ssf/rs staged on DVE behind quant-g0
            sweep_tile(0, 0, sumsA)
            ssf_grp(2, 4)
            sweep_tile(0, 1, sumsA)
            rs_batch(2, 4)
            ssf_grp(4, 8)
            sweep_tile(0, 2, sumsA)
            sweep_tile(0, 3, sumsA)
            rs_batch(4, 8)
            ssf_grp(8, 12)
            sweep_tile(0, 4, sumsA)
            sweep_tile(0, 5, sumsA)
            ssf_grp(12, 16)
            rs_batch(8, 16)
            sweep_tile(0, 6, sumsA)
            quant_grp(1)
            for b in range(7, NB):
                sweep_tile(0, b, sumsA)

            nc.sync.dma_start(ccA_in[:], sumsA[:])

            # DVE: rawdot products + reduces (wtgt landed ~25us)
            for t0 in range(0, NB, 4):
                pr = sp.tile([128, 4, D], BF16, tag="prod", name="gpr")
                nc.vector.tensor_mul(pr[:], fnat3[:, t0:t0 + 4, :],
                                     wtgt3[:, t0:t0 + 4, :])
                nc.vector.tensor_reduce(rawdot[:, t0:t0 + 4], pr[:],
                                        axis=AXX, op=ALU.add)

            # DVE: wn2 products + reduces (gp stays idle: its TENSOR_TENSOR
            # contends for the shared second SBUF port and slows DVE ~1.6x)
            for t0 in range(0, NB, 4):
                pr = sp.tile([128, 4, D], BF16, tag="gw", name="gwr")
                nc.vector.tensor_mul(pr[:], wtgt3[:, t0:t0 + 4, :],
                                     wtgt3[:, t0:t0 + 4, :])
                nc.vector.tensor_reduce(wn2[:, t0:t0 + 4], pr[:],
                                        axis=AXX, op=ALU.add)

            nc.gpsimd.collective_compute(
                "AllGather", ALU.bypass,
                replica_groups=[list(range(NCORES))],
                ins=[ccA_in[:].opt()],
                outs=[ccA_out[:].opt()],
            )

            for _ in range(8):
                nc.tensor.ldweights(ones128[:])

            last_exp = None
            for b in range(NB):
                last_exp = sweep_tile(1, b, sumsB)

            nc.sync.dma_start(ccB_in[:], sumsB[:])
            nc.gpsimd.collective_compute(
                "AllGather", ALU.bypass,
                replica_groups=[list(range(NCORES))],
                ins=[ccB_in[:].opt()],
                outs=[ccB_out[:].opt()],
            )
            gathA = pp.tile([128, NCORES, NB], F32)
            nc.sync.dma_start(
                gathA[:], ccA_out[:].rearrange("(g p) c -> p g c", p=128))
            gathB = pp.tile([128, NCORES, NB], F32)
            nc.sync.dma_start(
                gathB[:], ccB_out[:].rearrange("(g p) c -> p g c", p=128))

            fullsumA = pp.tile([128, NB], F32)
            nc.vector.tensor_reduce(
                fullsumA[:], gathA[:].rearrange("p g c -> p c g"),
                axis=AXX, op=ALU.add)
            fullsumB = pp.tile([128, NB], F32)
            nc.vector.tensor_reduce(
                fullsumB[:], gathB[:].rearrange("p g c -> p c g"),
                axis=AXX, op=ALU.add)

            # ---- combine (hidden under AG flights) ----
            m2 = pp.tile([128, NB], F32)
            nc.vector.tensor_mul(m2[:], ssf[:], wn2[:])
            lm2 = pp.tile([128, NB], F32)
            lm2_ln = nc.scalar.activation(lm2[:], m2[:], AF.Ln)
            add_dep_helper(lm2_ln.ins, last_exp.ins,
                           reason="combine ACT after sweep EXPs")
            rboth = pp.tile([128, NB], F32)
            nc.scalar.activation(rboth[:], lm2[:], AF.Exp, scale=-0.5)
            tgt = pp.tile([128, NB], F32)
            nc.vector.tensor_mul(tgt[:], rawdot[:], rboth[:])
            exptgt = pp.tile([128, NB], F32)
            nc.scalar.activation(exptgt[:], tgt[:], AF.Exp, scale=S)
            tclip = pp.tile([128, NB], F32)
            nc.vector.tensor_scalar(
                tclip[:], tgt[:], -1.0 + EPS, 1.0 - EPS,
                op0=ALU.max, op1=ALU.min)
            om = pp.tile([128, NB], F32)
            nc.vector.tensor_mul(om[:], tclip[:], tclip[:])
            nc.vector.tensor_scalar(om[:], om[:], -1.0, 1.0,
                                    op0=ALU.mult, op1=ALU.add)
            lom = pp.tile([128, NB], F32)
            nc.scalar.activation(lom[:], om[:], AF.Ln)
            snt = pp.tile([128, NB], F32)
            nc.scalar.activation(snt[:], lom[:], AF.Exp, scale=0.5)
            num = pp.tile([128, NB], F32)
            nc.vector.tensor_scalar_mul(num[:], tclip[:], S * COSM)
            snts = pp.tile([128, NB], F32)
            nc.vector.tensor_scalar_mul(snts[:], snt[:], S * SINM)
            nc.vector.tensor_sub(num[:], num[:], snts[:])
            expnum = pp.tile([128, NB], F32)
            nc.scalar.activation(expnum[:], num[:], AF.Exp)

            # ---- denominator chain ----
            fullsum = pp.tile([128, NB], F32)
            denom = pp.tile([128, NB], F32)
            logd = pp.tile([128, NB], F32)
            lvals = pp.tile([128, NB], F32)
            nc.vector.tensor_add(fullsum[:], fullsumA[:], fullsumB[:])
            nc.vector.tensor_add(denom[:], expnum[:], fullsum[:])
            nc.vector.tensor_sub(denom[:], denom[:], exptgt[:])
            nc.scalar.activation(logd[:], denom[:], AF.Ln)
            nc.vector.tensor_sub(lvals[:], num[:], logd[:])
            lred = pp.tile([128, 1], F32)
            nc.vector.reduce_sum(lred[:], lvals[:], axis=AXX)
            zf = pmain.tile([128, CW], F32, tag="z", name="zf")
            nc.tensor.matmul(zf[0:1, 0:1], ones_f32[:], lred[:],
                             start=True, stop=True)
            outv = pp.tile([1, 1], F32)
            nc.scalar.mul(outv[:], zf[0:1, 0:1], -1.0 / float(B))
            nc.sync.dma_start(out_ext[:], outv[:])
            pmain_cm.__exit__(None, None, None)

    nc.compile()
    return nc


def _prep_inputs(features, y_true, weight):
    features = np.asarray(features, dtype=np.float32)
    weight = np.asarray(weight, dtype=np.float32)
    y = np.asarray(y_true).astype(np.int64)

    fT = features.T.astype(FP8NP, order="C")           # [D, B] fp8
    fnat = features.astype(BF16NP)                     # [B, D] bf16
    wtgt = weight[y].astype(BF16NP)                    # [B, D] bf16

    in_maps = []
    for i in range(NCORES):
        shard = weight[i * CS:(i + 1) * CS]            # [CS, D]
        wT = shard.T.astype(BF16NP, order="C")         # [D, CS] bf16
        in_maps.append({"fT": fT, "wT": wT, "fnat": fnat, "wtgt": wtgt})
    return in_maps


def _run(features, y_true, weight, trace=False, **run_kwargs):
    if "nc" not in _CACHE:
        _CACHE["nc"] = _build()
    nc = _CACHE["nc"]
    in_maps = _prep_inputs(features, y_true, weight)
    res = run_bass_kernel_spmd(
        nc, in_maps, core_ids=list(range(NCORES)), trace=trace, **run_kwargs)
    out = np.asarray(res.results[0]["out"], dtype=np.float32)
    return np.float32(out.reshape(-1)[0]), res


def kernel(features, y_true, weight):
    val, _ = _run(features, y_true, weight, trace=False)
    return np.asarray(val, dtype=np.float32)
